# revision 9
# baseline (speedup 1.0000x reference)
"""Trainium2 Bass kernel for nn_AdvancedFuzzyAttention.

Math: softmax over rows that are constant along the key axis is exactly
uniform (1/S), so attention reduces to the per-batch mean of V broadcast
over queries; Q/K/fuzzy params never affect the output.

    valbar[b]  = mean_s value[b,s,:]
    obar[b]    = (valbar[b] @ Wv + bv) @ Wo + bo          (constant over s)
    gate[b]    = sigmoid(relu(obar[b] @ gW1 + gb1) @ gW2 + gb2)
    out[b,s]   = LN(query[b,s] + obar[b]*gate[b]) * ln_g + ln_b
    attn       = full(1/S)  (exact in fp32)

Distribution over 8 cores: batch b -> core c=b for value/query/LN;
weight matmuls sharded (Wv by columns, Wo by rows, gW1 by columns) with
an AllGather of valbar and AllReduces for obar and the gate logit.
Bias terms are folded on the host:  vconst = bv@Wo + bo,
gb1_eff = gb1 + vconst@gW1 (exact distributivity up to fp rounding).

Ring discipline: the SP (sync) HWDGE ring carries only wait-free
streaming loads and the final stores; every DMA that depends on a
collective lives on the gpsimd ring alongside the collective triggers,
so streaming never stalls behind a collective wait.
"""

from contextlib import ExitStack

import numpy as np

import concourse.bacc as bacc
import concourse.bass as bass
import concourse.tile as tile
from concourse import masks, mybir
from concourse.bass_utils import run_bass_kernel_spmd

N_CORES = 8
B, S, HID = 8, 512, 4096
P = 128
CS = HID // N_CORES          # 512: Wv col-shard / Wo row-shard per core
GS = (HID // 4) // N_CORES   # 128: gW1 col-shard per core
F32 = mybir.dt.float32
NK = HID // P                # 32 K-chunks of 128
NS = HID // CS               # 8 column chunks of 512
HALF = HID // 2


def _build():
    nc = bacc.Bacc(
        "TRN2", debug=False, target_bir_lowering=False, num_devices=N_CORES
    )

    value_b = nc.dram_tensor("value_b", [S, HID], F32, kind="ExternalInput")
    query_b = nc.dram_tensor("query_b", [S, HID], F32, kind="ExternalInput")
    wv_cs = nc.dram_tensor("wv_cs", [HID, CS], F32, kind="ExternalInput")
    wo_rs = nc.dram_tensor("wo_rs", [CS, HID], F32, kind="ExternalInput")
    g1_cs = nc.dram_tensor("g1_cs", [HID, GS], F32, kind="ExternalInput")
    gb1e = nc.dram_tensor("gb1e", [GS, 1], F32, kind="ExternalInput")
    gw2_cs = nc.dram_tensor("gw2_cs", [GS, 1], F32, kind="ExternalInput")
    gb2_f = nc.dram_tensor("gb2_f", [1, 1], F32, kind="ExternalInput")
    vconst = nc.dram_tensor("vconst", [1, HID], F32, kind="ExternalInput")
    ln_g = nc.dram_tensor("ln_g", [1, HID], F32, kind="ExternalInput")
    ln_b = nc.dram_tensor("ln_b", [1, HID], F32, kind="ExternalInput")
    bsel = nc.dram_tensor("bsel", [B, 1], F32, kind="ExternalInput")
    out_ext = nc.dram_tensor("out", [S, HID], F32, kind="ExternalOutput")

    def bcast(src, parts):
        # [1, N] DRAM access -> [parts, N] partition-broadcast AP
        a = src[:] if not isinstance(src, bass.AP) else src
        return bass.AP(tensor=a.tensor, offset=a.offset,
                       ap=[[0, parts]] + list(a.ap[1:]))

    with tile.TileContext(nc) as tc, ExitStack() as ctx:
        rg = [list(range(N_CORES))]
        pool = lambda **kw: ctx.enter_context(tc.tile_pool(**kw))

        dram = pool(name="dram", bufs=1, space="DRAM")
        ag_in = dram.tile([1, HID], F32)
        ag_out = dram.tile([B, HID], F32)
        ar_in = dram.tile([B, HID], F32)
        ar_out = dram.tile([B, HID], F32)
        l_in = dram.tile([B, 1], F32)
        l_out = dram.tile([B, 1], F32)
        og_dram = dram.tile([1, HID], F32)

        persist = pool(name="persist", bufs=1)
        bcastp = pool(name="bcastp", bufs=1)
        vp = pool(name="vp", bufs=2)       # value half-tiles [P, HALF]
        qp = pool(name="qp", bufs=3)       # query tiles [P, HID]
        wop = pool(name="wop", bufs=2)     # Wo half-tiles [P, HALF]
        wvp = pool(name="wvp", bufs=2)     # Wv tiles [P, CS]
        g1p = pool(name="g1p", bufs=32)    # gW1 tiles [P, GS], all resident
        ogp = pool(name="ogp", bufs=2)
        stg = pool(name="stg", bufs=2)
        statp = pool(name="statp", bufs=3)

        # single PSUM pool: every PSUM tile <= 1 bank; 8 slots total
        psp = pool(name="psp", bufs=8, space="PSUM")
        pst = lambda shape, name: psp.tile(shape, F32, tag="bank", name=name)

        # ---- constants + early wait-free loads ----
        identity = persist.tile([P, P], F32)
        masks.make_identity(nc, identity[:])
        ones_col = persist.tile([P, 1], F32)
        nc.vector.memset(ones_col[:], 1.0 / S)
        eps_sb = persist.tile([P, 1], F32)
        nc.vector.memset(eps_sb[:], 1e-5)
        bsel_sb = persist.tile([B, 1], F32)
        nc.sync.dma_start(out=bsel_sb[:], in_=bsel[:])
        gw2_sb = persist.tile([GS, 1], F32)
        nc.sync.dma_start(out=gw2_sb[:], in_=gw2_cs[:])
        gb1e_sb = persist.tile([GS, 1], F32)
        nc.sync.dma_start(out=gb1e_sb[:], in_=gb1e[:])
        vconst_sb = persist.tile([1, HID], F32)
        nc.sync.dma_start(out=vconst_sb[:], in_=vconst[:])
        # input-only broadcasts: issue first on the gpsimd ring
        gb2b_sb = persist.tile([B, 1], F32)
        nc.gpsimd.dma_start(out=gb2b_sb[:], in_=bcast(gb2_f, B))
        ln_gb = bcastp.tile([P, HID], F32)
        nc.gpsimd.dma_start(out=ln_gb[:], in_=bcast(ln_g, P))
        ln_bb = bcastp.tile([P, HID], F32)
        nc.gpsimd.dma_start(out=ln_bb[:], in_=bcast(ln_b, P))

        # ---- stage 1: valbar = mean_s value (scaled-ones matmul) ----
        # halves of the HID axis so only 4 accumulator banks live at once
        for h in range(2):
            ps_vb = [pst([1, CS], f"ps_vb{h}_{n}") for n in range(4)]
            for j in range(4):
                vt = vp.tile([P, HALF], F32, tag="v")
                nc.sync.dma_start(
                    out=vt[:],
                    in_=value_b[P * j:P * (j + 1), HALF * h:HALF * (h + 1)],
                )
                for n in range(4):
                    nc.tensor.matmul(
                        ps_vb[n][:], ones_col[:], vt[:, CS * n:CS * (n + 1)],
                        start=(j == 0), stop=(j == 3),
                    )
            for n in range(4):
                vbs = stg.tile([1, CS], F32, tag="vbs")
                nc.scalar.copy(out=vbs[:], in_=ps_vb[n][:])
                off = HALF * h + CS * n
                nc.gpsimd.dma_start(out=ag_in[:, off:off + CS], in_=vbs[:])

        # ---- stage 2: AllGather valbar -> VB [8, HID] ----
        nc.gpsimd.collective_compute(
            "AllGather", mybir.AluOpType.bypass, replica_groups=rg,
            ins=[ag_in[:].opt()], outs=[ag_out[:].opt()],
        )
        vb_sb = persist.tile([B, HID], F32, tag="mat8")
        nc.gpsimd.dma_start(out=vb_sb[:], in_=ag_out[:])

        # ---- stage 3: VBt chunks [128, 8] via PE transpose ----
        vbt_sb = persist.tile([P, NK * B], F32, tag="t8")
        for j in range(NK):
            tp = pst([P, B], f"tpv{j}")
            nc.tensor.transpose(
                tp[:], vb_sb[:B, P * j:P * (j + 1)], identity[:B, :B]
            )
            nc.scalar.copy(out=vbt_sb[:, B * j:B * (j + 1)], in_=tp[:])

        # ---- stage 4: vbarT[m] [128,8] = Wv_cs[:,m]T @ VB^T ----
        ps_vt = [pst([P, B], f"ps_vt{m}") for m in range(4)]
        for j in range(NK):
            wv = wvp.tile([P, CS], F32, tag="wv")
            nc.sync.dma_start(out=wv[:], in_=wv_cs[P * j:P * (j + 1), :])
            for m in range(4):
                nc.tensor.matmul(
                    ps_vt[m][:], wv[:, P * m:P * (m + 1)],
                    vbt_sb[:, B * j:B * (j + 1)],
                    start=(j == 0), stop=(j == NK - 1),
                )
        vbart_sb = persist.tile([P, 4 * B], F32)
        for m in range(4):
            nc.scalar.copy(out=vbart_sb[:, B * m:B * (m + 1)], in_=ps_vt[m][:])

        # ---- stage 5: obar_part [8, HID] = vbarT^T @ Wo_rs ----
        for h in range(2):
            ps_ob = [pst([B, CS], f"ps_ob{h}_{n}") for n in range(4)]
            for j in range(4):
                wo = wop.tile([P, HALF], F32, tag="wo")
                nc.sync.dma_start(
                    out=wo[:],
                    in_=wo_rs[P * j:P * (j + 1), HALF * h:HALF * (h + 1)],
                )
                for n in range(4):
                    nc.tensor.matmul(
                        ps_ob[n][:], vbart_sb[:, B * j:B * (j + 1)],
                        wo[:, CS * n:CS * (n + 1)],
                        start=(j == 0), stop=(j == 3),
                    )
            for n in range(4):
                obs = stg.tile([B, CS], F32, tag="obs")
                nc.scalar.copy(out=obs[:], in_=ps_ob[n][:])
                off = HALF * h + CS * n
                nc.gpsimd.dma_start(out=ar_in[:, off:off + CS], in_=obs[:])

        # ---- g1 loads: all 32 tiles resident, wait-free on the SP ring ----
        g1_tiles = []
        for j in range(NK):
            g1 = g1p.tile([P, GS], F32, tag="g1", name=f"g1_{j}")
            nc.sync.dma_start(out=g1[:], in_=g1_cs[P * j:P * (j + 1), :])
            g1_tiles.append(g1)

        # ---- query loads: hoisted, 3 slots for 4 tiles ----
        q_tiles = []
        for t in range(4):
            q = qp.tile([P, HID], F32, tag="q", name=f"q_{t}")
            nc.sync.dma_start(out=q[:], in_=query_b[P * t:P * (t + 1), :])
            q_tiles.append(q)

        # ---- stage 6: AllReduce obar ----
        nc.gpsimd.collective_compute(
            "AllReduce", mybir.AluOpType.add, replica_groups=rg,
            ins=[ar_in[:].opt()], outs=[ar_out[:].opt()],
        )
        obar_sb = persist.tile([B, HID], F32, tag="mat8")
        nc.gpsimd.dma_start(out=obar_sb[:], in_=ar_out[:])

        # ---- stage 7: gate ----
        obart_sb = persist.tile([P, NK * B], F32, tag="t8")
        for j in range(NK):
            tp = pst([P, B], f"tpo{j}")
            nc.tensor.transpose(
                tp[:], obar_sb[:B, P * j:P * (j + 1)], identity[:B, :B]
            )
            nc.scalar.copy(out=obart_sb[:, B * j:B * (j + 1)], in_=tp[:])

        ps_h = pst([GS, B], "ps_h")
        for j in range(NK):
            nc.tensor.matmul(
                ps_h[:], g1_tiles[j][:], obart_sb[:, B * j:B * (j + 1)],
                start=(j == 0), stop=(j == NK - 1),
            )
        ht_sb = persist.tile([GS, B], F32)
        nc.scalar.activation(
            out=ht_sb[:], in_=ps_h[:],
            func=mybir.ActivationFunctionType.Relu, bias=gb1e_sb[:], scale=1.0,
        )
        ps_l = pst([B, 1], "ps_l")
        nc.tensor.matmul(ps_l[:], ht_sb[:], gw2_sb[:], start=True, stop=True)
        l_sb = persist.tile([B, 1], F32)
        nc.scalar.copy(out=l_sb[:], in_=ps_l[:])
        nc.gpsimd.dma_start(out=l_in[:], in_=l_sb[:])

        nc.gpsimd.collective_compute(
            "AllReduce", mybir.AluOpType.add, replica_groups=rg,
            ins=[l_in[:].opt()], outs=[l_out[:].opt()],
        )
        lsum_sb = persist.tile([B, 1], F32)
        nc.gpsimd.dma_start(out=lsum_sb[:], in_=l_out[:])
        gate_sb = persist.tile([B, 1], F32)
        nc.scalar.activation(
            out=gate_sb[:], in_=lsum_sb[:],
            func=mybir.ActivationFunctionType.Sigmoid,
            bias=gb2b_sb[:], scale=1.0,
        )
        ps_gc = pst([1, 1], "ps_gc")
        nc.tensor.matmul(ps_gc[:], bsel_sb[:], gate_sb[:], start=True, stop=True)
        gatec_sb = persist.tile([1, 1], F32)
        nc.scalar.copy(out=gatec_sb[:], in_=ps_gc[:])

        # ---- stage 8: og_c = (obar_c + vconst) * gate_c -> og_dram ----
        for n in range(NS):
            ps_s = pst([1, CS], f"ps_s{n}")
            nc.tensor.matmul(
                ps_s[:], bsel_sb[:], obar_sb[:B, CS * n:CS * (n + 1)],
                start=True, stop=True,
            )
            ogt = ogp.tile([1, CS], F32, tag="og")
            nc.vector.tensor_add(
                ogt[:], ps_s[:], vconst_sb[:, CS * n:CS * (n + 1)]
            )
            nc.vector.tensor_scalar_mul(ogt[:], ogt[:], gatec_sb[:])
            nc.gpsimd.dma_start(out=og_dram[:, CS * n:CS * (n + 1)], in_=ogt[:])

        ogb = bcastp.tile([P, HID], F32)
        nc.gpsimd.dma_start(out=ogb[:], in_=bcast(og_dram, P))

        # ---- stage 9: LayerNorm(query + og) ----
        for t in range(4):
            q = q_tiles[t]
            nc.gpsimd.tensor_add(q[:], q[:], ogb[:])
            st = statp.tile([P, NS, 6], F32, tag="st")
            for sg in range(NS):
                nc.vector.bn_stats(
                    out=st[:, sg, :], in_=q[:, CS * sg:CS * (sg + 1)]
                )
            mv = statp.tile([P, 2], F32, tag="mv")
            nc.vector.bn_aggr(out=mv[:], in_=st[:])
            # rstd = 1/sqrt(var + eps)
            nc.scalar.activation(
                out=mv[:, 1:2], in_=mv[:, 1:2],
                func=mybir.ActivationFunctionType.Sqrt,
                bias=eps_sb[:], scale=1.0,
            )
            nc.vector.reciprocal(out=mv[:, 1:2], in_=mv[:, 1:2])
            # nb = -mean * rstd, then one ACT pass: q*rstd + nb
            nb = statp.tile([P, 1], F32, tag="nb")
            nc.scalar.activation(
                out=nb[:], in_=mv[:, 0:1],
                func=mybir.ActivationFunctionType.Copy, scale=-1.0,
            )
            nc.vector.tensor_mul(nb[:], nb[:], mv[:, 1:2])
            nc.scalar.activation(
                out=q[:], in_=q[:],
                func=mybir.ActivationFunctionType.Identity,
                bias=nb[:], scale=mv[:, 1:2],
            )
            nc.vector.tensor_mul(q[:], q[:], ln_gb[:])
            nc.vector.tensor_add(q[:], q[:], ln_bb[:])
            nc.sync.dma_start(out=out_ext[P * t:P * (t + 1), :], in_=q[:])

    nc.compile()
    return nc


_NC = None


def _get_nc():
    global _NC
    if _NC is None:
        _NC = _build()
    return _NC


def _make_in_maps(inputs):
    f = lambda k: np.ascontiguousarray(np.asarray(inputs[k], np.float32))
    value, query = f("value"), f("query")
    Wv, Wo = f("Wv"), f("Wo")
    gW1, gW2 = f("gW1"), f("gW2")
    bv, bo, gb1, gb2 = f("bv"), f("bo"), f("gb1"), f("gb2")
    ln_g, ln_b = f("ln_g"), f("ln_b")

    vconst = (bv @ Wo + bo).astype(np.float32)          # [HID]
    gb1_eff = (gb1 + vconst @ gW1).astype(np.float32)   # [HID/4]

    in_maps = []
    for c in range(N_CORES):
        sel = np.zeros((B, 1), np.float32)
        sel[c, 0] = 1.0
        in_maps.append({
            "value_b": value[c],
            "query_b": query[c],
            "wv_cs": np.ascontiguousarray(Wv[:, CS * c:CS * (c + 1)]),
            "wo_rs": np.ascontiguousarray(Wo[CS * c:CS * (c + 1), :]),
            "g1_cs": np.ascontiguousarray(gW1[:, GS * c:GS * (c + 1)]),
            "gb1e": np.ascontiguousarray(
                gb1_eff[GS * c:GS * (c + 1)].reshape(GS, 1)),
            "gw2_cs": np.ascontiguousarray(gW2[GS * c:GS * (c + 1), :]),
            "gb2_f": gb2.reshape(1, 1),
            "vconst": vconst.reshape(1, HID),
            "ln_g": ln_g.reshape(1, HID),
            "ln_b": ln_b.reshape(1, HID),
            "bsel": sel,
        })
    return in_maps


def _execute(inputs, trace=False):
    nc = _get_nc()
    res = run_bass_kernel_spmd(
        nc, _make_in_maps(inputs), core_ids=list(range(N_CORES)), trace=trace
    )
    out = np.stack([res.results[c]["out"] for c in range(N_CORES)], axis=0)
    attn = np.full((B, 8, S, S), np.float32(1.0 / S), np.float32)
    return (out.astype(np.float32), attn), res


def kernel(**inputs):
    outs, _ = _execute(inputs, trace=False)
    return outs


# ---------------------------------------------------------------------------
# Benchmark path: cached jitted PJRT callable; HW time estimated from the
# slope of pipelined async executions (cancels per-call dispatch latency).
# ---------------------------------------------------------------------------
_RUNNER = None


def _get_runner():
    global _RUNNER
    if _RUNNER is not None:
        return _RUNNER
    import jax
    from jax.experimental.shard_map import shard_map
    from jax.sharding import Mesh, PartitionSpec

    from concourse import bass2jax

    bass2jax.install_neuronx_cc_hook()
    nc = _get_nc()
    partition_name = (
        nc.partition_id_tensor.name if nc.partition_id_tensor else None
    )
    in_names, out_names, out_avals = [], [], []
    for alloc in nc.m.functions[0].allocations:
        if not isinstance(alloc, mybir.MemoryLocationSet):
            continue
        name = alloc.memorylocations[0].name
        if alloc.kind == "ExternalInput":
            if name != partition_name:
                in_names.append(name)
        elif alloc.kind == "ExternalOutput":
            out_names.append(name)
            out_avals.append(
                jax.core.ShapedArray(
                    tuple(alloc.tensor_shape), mybir.dt.np(alloc.dtype)
                )
            )
    n_params = len(in_names)
    all_names = in_names + out_names + (
        [partition_name] if partition_name else []
    )

    def _body(*args):
        operands = list(args)
        if partition_name is not None:
            operands.append(bass2jax.partition_id_tensor())
        return tuple(
            bass2jax._bass_exec_p.bind(
                *operands,
                out_avals=tuple(out_avals),
                in_names=tuple(all_names),
                out_names=tuple(out_names),
                lowering_input_output_aliases=(),
                sim_require_finite=True,
                sim_require_nnan=True,
                nc=nc,
            )
        )

    devices = jax.devices()[:N_CORES]
    mesh = Mesh(np.asarray(devices), ("core",))
    nin = n_params + len(out_names)
    fn = jax.jit(
        shard_map(
            _body,
            mesh=mesh,
            in_specs=(PartitionSpec("core"),) * nin,
            out_specs=(PartitionSpec("core"),) * len(out_names),
            check_rep=False,
        ),
        keep_unused=True,
    )
    _RUNNER = (fn, in_names, out_names, out_avals, mesh)
    return _RUNNER


def bench(inputs, iters=16):
    import time

    import jax
    from jax.sharding import NamedSharding, PartitionSpec

    fn, in_names, out_names, out_avals, mesh = _get_runner()
    in_maps = _make_in_maps(inputs)
    sh = NamedSharding(mesh, PartitionSpec("core"))
    args = []
    for name in in_names:
        arr = np.concatenate(
            [np.asarray(in_maps[c][name]) for c in range(N_CORES)], axis=0
        )
        args.append(jax.device_put(arr, sh))
    for av in out_avals:
        z = np.zeros((N_CORES * av.shape[0], *av.shape[1:]), av.dtype)
        args.append(jax.device_put(z, sh))

    outs = fn(*args)
    jax.block_until_ready(outs)  # compile + warmup

    singles = []
    for _ in range(5):
        t0 = time.perf_counter()
        jax.block_until_ready(fn(*args))
        singles.append(time.perf_counter() - t0)
    t_single = min(singles)

    t0 = time.perf_counter()
    o = None
    for _ in range(iters):
        o = fn(*args)
    jax.block_until_ready(o)
    t_n = time.perf_counter() - t0
    slope = (t_n - t_single) / (iters - 1)

    out_g = np.asarray(outs[out_names.index("out")])
    out = out_g.reshape(N_CORES, S, HID)
    attn = np.full((B, 8, S, S), np.float32(1.0 / S), np.float32)
    return (out.astype(np.float32), attn), t_single, slope
